# revision 6
# baseline (speedup 1.0000x reference)
"""Dynamic per-pixel depthwise 3x3 conv (DYDConv2d) on 8 Trainium2 cores.

Full-tensor contract:
    input : (8, 64, 128, 128) f32
    weight: (8, 64, 3, 3, 128, 128) f32   -- one 3x3 filter per (b, c, oh, ow)
    out   : (8, 64, 128, 128) f32
    out[b,c,oh,ow] = sum_{i,j} xpad[b,c,oh+i,ow+j] * weight[b,c,i,j,oh,ow]
    (stride 1, pad 1)

Sharding: data-parallel over batch B=8 -> one sample per NeuronCore.

Per-core layout: 128 SBUF partitions = (channel c in 0..63) x (H-half hf in
{0,1}), partition p = c*2 + hf.  Each partition holds a (66 x 130) zero-
padded slab of its half-image (64 rows + 1-row halo on each side, 128 cols +
1-col pad on each side).  The slab (pads included) is assembled on the host
and shipped as a [128, 66*130] tensor so it loads with one contiguous
full-bandwidth DMA; the 9 taps are then shifted free-dim views of the slab.
Output rows are processed in chunks of RT rows; per chunk, 9 per-tap DMAs
bring the weight planes ([128, RT, 128] each), the 9 multiplies run on the
vector engine, and the 8 accumulate-adds are split between the vector engine
and GPSIMD (NA adds on GPSIMD) so elementwise work hides under the weight
stream, which is the roofline term (~38 MB/core of weights).
"""

import os

import numpy as np

import concourse.bacc as bacc
import concourse.mybir as mybir
from concourse.bass_utils import run_bass_kernel_spmd
from concourse.tile import TileContext

B, C, H, W = 8, 64, 128, 128
KH, KW = 3, 3
HALF = H // 2  # rows per half-image (one partition group)
SLAB_R, SLAB_C = HALF + 2, W + 2  # 66 x 130 padded slab per partition

RT = int(os.environ.get("DYD_RT", "8"))  # output rows per chunk (per half)
NA = int(os.environ.get("DYD_NA", "6"))  # accumulate-adds run on GPSIMD (0..6)
N_CHUNKS = HALF // RT

_F32 = mybir.dt.float32


def _emit(nc, tc, xs, w, o):
    """Per-core program. xs:[128, 66*130] w:[64,3,3,128,128] o:[64,128,128]."""
    # DRAM views with partition layout p = c*2 + hf.
    wv = w.rearrange("c kh kw (hf r) ww -> c hf (kh kw) r ww", hf=2)
    ov = o.rearrange("c (hf r) ww -> (c hf) r ww", hf=2)

    with tc.tile_pool(name="xpool", bufs=1) as xpool:
        xbuf = xpool.tile([128, SLAB_R, SLAB_C], _F32, name="xbuf")
        xpool.seal()
        nc.scalar.dma_start(
            out=xbuf[:].rearrange("p r cc -> p (r cc)"), in_=xs[:]
        )

        with tc.tile_pool(name="work", bufs=2) as pool:
            for k in range(N_CHUNKS):
                r0 = k * RT
                wtiles = []
                for t in range(KH * KW):
                    wt = pool.tile([128, RT, W], _F32, name=f"wt{t}")
                    nc.sync.dma_start(out=wt[:], in_=wv[:, :, t, r0 : r0 + RT, :])
                    wtiles.append(wt)

                def xtap(t):
                    i, j = divmod(t, KW)
                    return xbuf[:, r0 + i : r0 + i + RT, j : j + W]

                # Taps 0..NA-1 are multiplied on DVE but summed on GPSIMD;
                # taps NA..8 multiply and accumulate on DVE.
                pool_taps = list(range(NA))
                dve_taps = list(range(NA, KH * KW))

                # Products destined for GPSIMD first, so it can start early.
                prods = []
                for t in pool_taps:
                    p_t = pool.tile([128, RT, W], _F32, name=f"prod{t}")
                    nc.vector.tensor_tensor(
                        p_t[:], xtap(t), wtiles[t][:], mybir.AluOpType.mult
                    )
                    prods.append(p_t)

                accp = None
                if NA >= 2:
                    accp = pool.tile([128, RT, W], _F32, name="accp")
                    nc.gpsimd.tensor_tensor(
                        accp[:], prods[0][:], prods[1][:], mybir.AluOpType.add
                    )
                    for t in range(2, NA):
                        nc.gpsimd.tensor_tensor(
                            accp[:], accp[:], prods[t][:], mybir.AluOpType.add
                        )
                elif NA == 1:
                    accp = prods[0]

                # DVE chain over its own taps.
                accd = pool.tile([128, RT, W], _F32, name="accd")
                t0 = dve_taps[0]
                nc.vector.tensor_tensor(
                    accd[:], xtap(t0), wtiles[t0][:], mybir.AluOpType.mult
                )
                tmp = pool.tile([128, RT, W], _F32, name="tmp")
                for t in dve_taps[1:]:
                    nc.vector.tensor_tensor(
                        tmp[:], xtap(t), wtiles[t][:], mybir.AluOpType.mult
                    )
                    nc.vector.tensor_tensor(
                        accd[:], accd[:], tmp[:], mybir.AluOpType.add
                    )

                # Combine and store.
                if accp is not None:
                    out_t = accp
                    nc.gpsimd.tensor_tensor(
                        out_t[:], accp[:], accd[:], mybir.AluOpType.add
                    )
                else:
                    out_t = accd
                nc.scalar.dma_start(out=ov[:, r0 : r0 + RT, :], in_=out_t[:])


def build_program():
    nc = bacc.Bacc(
        "TRN2",
        target_bir_lowering=False,
        debug=False,
        enable_asserts=False,
        num_devices=8,
    )
    xs = nc.dram_tensor("xs", [128, SLAB_R * SLAB_C], _F32, kind="ExternalInput").ap()
    w = nc.dram_tensor("w", [C, KH, KW, H, W], _F32, kind="ExternalInput").ap()
    o = nc.dram_tensor("o", [C, H, W], _F32, kind="ExternalOutput").ap()
    with TileContext(nc) as tc:
        _emit(nc, tc, xs, w, o)
    nc.compile()
    return nc


def make_slab(x_one):
    """Host-side zero-padded slab for one sample: [64,128,128] -> [128, 66*130].

    Partition p = c*2 + hf holds rows hf*64-1 .. hf*64+64 of channel c
    (zero-padded at the image border) in a 66x130 col-padded layout.
    """
    slab = np.zeros((C, 2, SLAB_R, SLAB_C), dtype=np.float32)
    # half 0: slab rows 1..65 <- x rows 0..64 (row 0 stays zero: top pad)
    slab[:, 0, 1 : HALF + 2, 1 : W + 1] = x_one[:, 0 : HALF + 1, :]
    # half 1: slab rows 0..64 <- x rows 63..127 (row 65 stays zero: bottom pad)
    slab[:, 1, 0 : HALF + 1, 1 : W + 1] = x_one[:, HALF - 1 : H, :]
    return slab.reshape(128, SLAB_R * SLAB_C)


_CACHE = {}


def kernel(input, weight, _trace=False):
    input = np.asarray(input, dtype=np.float32)
    weight = np.asarray(weight, dtype=np.float32)
    assert input.shape == (B, C, H, W), input.shape
    assert weight.shape == (B, C, KH, KW, H, W), weight.shape

    if "nc" not in _CACHE:
        _CACHE["nc"] = build_program()
    nc = _CACHE["nc"]

    in_maps = [
        {"xs": make_slab(input[b]), "w": np.ascontiguousarray(weight[b])}
        for b in range(B)
    ]
    res = run_bass_kernel_spmd(nc, in_maps, core_ids=list(range(B)), trace=_trace)
    _CACHE["last_result"] = res
    out = np.stack([res.results[b]["o"] for b in range(B)], axis=0)
    return out.astype(np.float32, copy=False)


# revision 14
# speedup vs baseline: 69803.9354x; 69803.9354x over previous
"""Dynamic per-pixel depthwise 3x3 conv (DYDConv2d) on 8 Trainium2 cores.

Full-tensor contract:
    input : (8, 64, 128, 128) f32
    weight: (8, 64, 3, 3, 128, 128) f32   -- one 3x3 filter per (b, c, oh, ow)
    out   : (8, 64, 128, 128) f32
    out[b,c,oh,ow] = sum_{i,j} xpad[b,c,oh+i,ow+j] * weight[b,c,i,j,oh,ow]
    (stride 1, pad 1)

Sharding: data-parallel over batch B=8 -> one sample per NeuronCore.

Per-core layout: 128 SBUF partitions = (channel c in 0..63) x (H-half hf in
{0,1}), partition p = c*2 + hf.  Each partition holds a (66 x 130) zero-
padded slab of its half-image (64 rows + 1-row halo on each side, 128 cols +
1-col pad on each side).  The slab (pads included) is assembled on the host
and shipped as a [128, 66*130] tensor so it loads with one contiguous
full-bandwidth DMA; the 9 taps are then shifted free-dim views of the slab.
Output rows are processed in chunks of RT=16 rows; per chunk, 9 per-tap
1 MB DMAs bring the weight planes ([128, RT, 128] each) and the 9 multiplies
plus 8 accumulate-adds all run on the vector engine (measured fastest:
NA=0 -- GPSIMD "help" loses to its SBUF-port lock against DVE).  The weight
stream (~38 MB/core) is the roofline term; measured steady state is at the
DMA fabric rate (~90-110 us/core/pass vs a 106 us 436 GB/s floor).
"""

import numpy as np

import concourse.bacc as bacc
import concourse.mybir as mybir
from concourse.bass_utils import run_bass_kernel_spmd
from concourse.tile import TileContext

B, C, H, W = 8, 64, 128, 128
KH, KW = 3, 3
HALF = H // 2  # rows per half-image (one partition group)
SLAB_R, SLAB_C = HALF + 2, W + 2  # 66 x 130 padded slab per partition

RT = 16  # output rows per chunk (per half): 4 chunks, 1 MB per-tap DMAs
NA = 0   # accumulate-adds on GPSIMD; 0 = all-DVE (fastest: no port-lock loss)
N_CHUNKS = HALF // RT

_F32 = mybir.dt.float32


def _emit(nc, tc, xs, w, o, rep=1, na=None, mode="full", rt=None, split=False):
    """Per-core program. xs:[128, 66*130] w:[64,3,3,128,128] o:[64,128,128].

    rep > 1 repeats the complete pass (x load included) back-to-back in one
    program — used only for steady-state timing via differencing.
    """
    # DRAM views with partition layout p = c*2 + hf.
    wv = w.rearrange("c kh kw (hf r) ww -> c hf (kh kw) r ww", hf=2)
    ov = o.rearrange("c (hf r) ww -> (c hf) r ww", hf=2)

    with tc.tile_pool(name="xpool", bufs=1) as xpool:
        xbuf = xpool.tile([128, SLAB_R, SLAB_C], _F32, name="xbuf")
        xpool.seal()

        with tc.tile_pool(name="work", bufs=2) as pool:
            for _r in range(rep):
                if mode != "compute":
                    nc.scalar.dma_start(
                        out=xbuf[:].rearrange("p r cc -> p (r cc)"), in_=xs[:]
                    )
                _emit_pass(
                    nc, pool, xbuf, wv, ov, na=na, mode=mode, rt=rt, split=split
                )


def _emit_pass(nc, pool, xbuf, wv, ov, na=None, mode="full", rt=None, split=False):
    na = NA if na is None else na
    rt = RT if rt is None else rt
    for k in range(HALF // rt):
        r0 = k * rt
        wtiles = []
        for t in range(KH * KW):
            wt = pool.tile([128, rt, W], _F32, name=f"wt{t}")
            if mode != "compute":
                eng = nc.scalar if (split and t % 2 == 1) else nc.sync
                eng.dma_start(out=wt[:], in_=wv[:, :, t, r0 : r0 + rt, :])
            wtiles.append(wt)

        def xtap(t):
            i, j = divmod(t, KW)
            return xbuf[:, r0 + i : r0 + i + rt, j : j + W]

        # Taps 0..na-1 are multiplied on DVE but summed on GPSIMD;
        # taps na..8 multiply and accumulate on DVE.
        if mode == "dma":
            nc.scalar.dma_start(
                out=ov[:, r0 : r0 + rt, :],
                in_=xbuf[:, r0 : r0 + rt, 1 : W + 1],
            )
            continue
        pool_taps = list(range(na))
        dve_taps = list(range(na, KH * KW))

        # Products destined for GPSIMD first, so it can start early.
        prods = []
        for t in pool_taps:
            p_t = pool.tile([128, rt, W], _F32, name=f"prod{t}")
            nc.vector.tensor_tensor(
                p_t[:], xtap(t), wtiles[t][:], mybir.AluOpType.mult
            )
            prods.append(p_t)

        accp = None
        if na >= 2:
            accp = pool.tile([128, rt, W], _F32, name="accp")
            nc.gpsimd.tensor_tensor(
                accp[:], prods[0][:], prods[1][:], mybir.AluOpType.add
            )
            for t in range(2, na):
                nc.gpsimd.tensor_tensor(
                    accp[:], accp[:], prods[t][:], mybir.AluOpType.add
                )
        elif na == 1:
            accp = prods[0]

        # DVE chain over its own taps.
        accd = pool.tile([128, rt, W], _F32, name="accd")
        t0 = dve_taps[0]
        nc.vector.tensor_tensor(
            accd[:], xtap(t0), wtiles[t0][:], mybir.AluOpType.mult
        )
        tmp = pool.tile([128, rt, W], _F32, name="tmp", bufs=1)
        for t in dve_taps[1:]:
            nc.vector.tensor_tensor(
                tmp[:], xtap(t), wtiles[t][:], mybir.AluOpType.mult
            )
            nc.vector.tensor_tensor(
                accd[:], accd[:], tmp[:], mybir.AluOpType.add
            )

        # Combine and store.
        if accp is not None:
            out_t = accp
            nc.gpsimd.tensor_tensor(
                out_t[:], accp[:], accd[:], mybir.AluOpType.add
            )
        else:
            out_t = accd
        nc.scalar.dma_start(out=ov[:, r0 : r0 + rt, :], in_=out_t[:])




def build_program(rep=1, na=None, mode="full", rt=None, split=False):
    nc = bacc.Bacc(
        "TRN2",
        target_bir_lowering=False,
        debug=False,
        enable_asserts=False,
        num_devices=8,
    )
    xs = nc.dram_tensor("xs", [128, SLAB_R * SLAB_C], _F32, kind="ExternalInput").ap()
    w = nc.dram_tensor("w", [C, KH, KW, H, W], _F32, kind="ExternalInput").ap()
    o = nc.dram_tensor("o", [C, H, W], _F32, kind="ExternalOutput").ap()
    with TileContext(nc) as tc:
        _emit(nc, tc, xs, w, o, rep=rep, na=na, mode=mode, rt=rt, split=split)
    nc.compile()
    return nc


def make_slab(x_one):
    """Host-side zero-padded slab for one sample: [64,128,128] -> [128, 66*130].

    Partition p = c*2 + hf holds rows hf*64-1 .. hf*64+64 of channel c
    (zero-padded at the image border) in a 66x130 col-padded layout.
    """
    slab = np.zeros((C, 2, SLAB_R, SLAB_C), dtype=np.float32)
    # half 0: slab rows 1..65 <- x rows 0..64 (row 0 stays zero: top pad)
    slab[:, 0, 1 : HALF + 2, 1 : W + 1] = x_one[:, 0 : HALF + 1, :]
    # half 1: slab rows 0..64 <- x rows 63..127 (row 65 stays zero: bottom pad)
    slab[:, 1, 0 : HALF + 1, 1 : W + 1] = x_one[:, HALF - 1 : H, :]
    return slab.reshape(128, SLAB_R * SLAB_C)


_CACHE = {}


def kernel(input, weight, _trace=False):
    input = np.asarray(input, dtype=np.float32)
    weight = np.asarray(weight, dtype=np.float32)
    assert input.shape == (B, C, H, W), input.shape
    assert weight.shape == (B, C, KH, KW, H, W), weight.shape

    if "nc" not in _CACHE:
        _CACHE["nc"] = build_program()
    nc = _CACHE["nc"]

    in_maps = [
        {"xs": make_slab(input[b]), "w": np.ascontiguousarray(weight[b])}
        for b in range(B)
    ]
    res = run_bass_kernel_spmd(nc, in_maps, core_ids=list(range(B)), trace=_trace)
    _CACHE["last_result"] = res
    out = np.stack([res.results[b]["o"] for b in range(B)], axis=0)
    return out.astype(np.float32, copy=False)



# revision 16
# speedup vs baseline: 82781.1006x; 1.1859x over previous
"""Dynamic per-pixel depthwise 3x3 conv (DYDConv2d) on 8 Trainium2 cores.

Full-tensor contract:
    input : (8, 64, 128, 128) f32
    weight: (8, 64, 3, 3, 128, 128) f32   -- one 3x3 filter per (b, c, oh, ow)
    out   : (8, 64, 128, 128) f32
    out[b,c,oh,ow] = sum_{i,j} xpad[b,c,oh+i,ow+j] * weight[b,c,i,j,oh,ow]
    (stride 1, pad 1)

Sharding: data-parallel over batch B=8 -> one sample per NeuronCore.

Per-core layout: 128 SBUF partitions = (channel c in 0..63) x (H-half hf in
{0,1}), partition p = c*2 + hf.  Each partition holds a (66 x 130) zero-
padded slab of its half-image (64 rows + 1-row halo on each side, 128 cols +
1-col pad on each side).  The slab (pads included) is assembled on the host
and shipped as a [128, 66*130] tensor so it loads with one contiguous
full-bandwidth DMA; the 9 taps are then shifted free-dim views of the slab.
Output rows are processed in 32-row chunks with taps streamed in groups of
3: per group, three 2 MB weight DMAs land double-buffered and the vector
engine runs the multiplies and accumulate-adds (FD=4096 per op).  All
elementwise work stays on the vector engine — offloading adds to GPSIMD
measured slower (shared SBUF-port lock).  The ~38 MB/core weight stream is
the roofline term; measured steady state ~91 us/core/pass, at the DMA
fabric-rate floor.
"""

import numpy as np

import concourse.bacc as bacc
import concourse.mybir as mybir
from concourse.bass_utils import run_bass_kernel_spmd
from concourse.tile import TileContext

B, C, H, W = 8, 64, 128, 128
KH, KW = 3, 3
HALF = H // 2  # rows per half-image (one partition group)
SLAB_R, SLAB_C = HALF + 2, W + 2  # 66 x 130 padded slab per partition

RT = 16  # output rows per chunk (per half): 4 chunks, 1 MB per-tap DMAs
NA = 0   # accumulate-adds on GPSIMD; 0 = all-DVE (fastest: no port-lock loss)
N_CHUNKS = HALF // RT

_F32 = mybir.dt.float32


def _emit(nc, tc, xs, w, o, rep=1, na=None, mode="full", rt=None, split=False):
    """Per-core program. xs:[128, 66*130] w:[64,3,3,128,128] o:[64,128,128].

    rep > 1 repeats the complete pass (x load included) back-to-back in one
    program — used only for steady-state timing via differencing.
    """
    # DRAM views with partition layout p = c*2 + hf.
    wv = w.rearrange("c kh kw (hf r) ww -> c hf (kh kw) r ww", hf=2)
    ov = o.rearrange("c (hf r) ww -> (c hf) r ww", hf=2)

    with tc.tile_pool(name="xpool", bufs=1) as xpool:
        xbuf = xpool.tile([128, SLAB_R, SLAB_C], _F32, name="xbuf")
        xpool.seal()

        with tc.tile_pool(name="work", bufs=2) as pool:
            for _r in range(rep):
                if mode != "compute":
                    nc.scalar.dma_start(
                        out=xbuf[:].rearrange("p r cc -> p (r cc)"), in_=xs[:]
                    )
                if mode == "g32":
                    _emit_pass_grouped(nc, pool, xbuf, wv, ov)
                else:
                    _emit_pass(
                        nc, pool, xbuf, wv, ov,
                        na=na, mode=mode, rt=rt, split=split,
                    )


def _emit_pass(nc, pool, xbuf, wv, ov, na=None, mode="full", rt=None, split=False):
    na = NA if na is None else na
    rt = RT if rt is None else rt
    for k in range(HALF // rt):
        r0 = k * rt
        wtiles = []
        for t in range(KH * KW):
            wt = pool.tile([128, rt, W], _F32, name=f"wt{t}")
            if mode != "compute":
                eng = nc.scalar if (split and t % 2 == 1) else nc.sync
                eng.dma_start(out=wt[:], in_=wv[:, :, t, r0 : r0 + rt, :])
            wtiles.append(wt)

        def xtap(t):
            i, j = divmod(t, KW)
            return xbuf[:, r0 + i : r0 + i + rt, j : j + W]

        # Taps 0..na-1 are multiplied on DVE but summed on GPSIMD;
        # taps na..8 multiply and accumulate on DVE.
        if mode == "dma":
            nc.scalar.dma_start(
                out=ov[:, r0 : r0 + rt, :],
                in_=xbuf[:, r0 : r0 + rt, 1 : W + 1],
            )
            continue
        pool_taps = list(range(na))
        dve_taps = list(range(na, KH * KW))

        # Products destined for GPSIMD first, so it can start early.
        prods = []
        for t in pool_taps:
            p_t = pool.tile([128, rt, W], _F32, name=f"prod{t}")
            nc.vector.tensor_tensor(
                p_t[:], xtap(t), wtiles[t][:], mybir.AluOpType.mult
            )
            prods.append(p_t)

        accp = None
        if na >= 2:
            accp = pool.tile([128, rt, W], _F32, name="accp")
            nc.gpsimd.tensor_tensor(
                accp[:], prods[0][:], prods[1][:], mybir.AluOpType.add
            )
            for t in range(2, na):
                nc.gpsimd.tensor_tensor(
                    accp[:], accp[:], prods[t][:], mybir.AluOpType.add
                )
        elif na == 1:
            accp = prods[0]

        # DVE chain over its own taps.
        accd = pool.tile([128, rt, W], _F32, name="accd")
        t0 = dve_taps[0]
        nc.vector.tensor_tensor(
            accd[:], xtap(t0), wtiles[t0][:], mybir.AluOpType.mult
        )
        tmp = pool.tile([128, rt, W], _F32, name="tmp", bufs=1)
        for t in dve_taps[1:]:
            nc.vector.tensor_tensor(
                tmp[:], xtap(t), wtiles[t][:], mybir.AluOpType.mult
            )
            nc.vector.tensor_tensor(
                accd[:], accd[:], tmp[:], mybir.AluOpType.add
            )

        # Combine and store.
        if accp is not None:
            out_t = accp
            nc.gpsimd.tensor_tensor(
                out_t[:], accp[:], accd[:], mybir.AluOpType.add
            )
        else:
            out_t = accd
        nc.scalar.dma_start(out=ov[:, r0 : r0 + rt, :], in_=out_t[:])




def _emit_pass_grouped(nc, pool, xbuf, wv, ov, rt=32, grp=3):
    """32-row chunks, taps streamed in groups of `grp`: 2 MB weight DMAs,
    FD=4096 DVE ops.  Weight residency = 2*grp tiles (double-buffered)."""
    for k in range(HALF // rt):
        r0 = k * rt

        def xtap(t):
            i, j = divmod(t, KW)
            return xbuf[:, r0 + i : r0 + i + rt, j : j + W]

        acc = pool.tile([128, rt, W], _F32, name="acc")
        tmp = pool.tile([128, rt, W], _F32, name="tmp", bufs=1)
        first = True
        for g0 in range(0, KH * KW, grp):
            wts = []
            for t in range(g0, min(g0 + grp, KH * KW)):
                wt = pool.tile([128, rt, W], _F32, name=f"wg{t - g0}")
                nc.sync.dma_start(out=wt[:], in_=wv[:, :, t, r0 : r0 + rt, :])
                wts.append((t, wt))
            for t, wt in wts:
                if first:
                    nc.vector.tensor_tensor(
                        acc[:], xtap(t), wt[:], mybir.AluOpType.mult
                    )
                    first = False
                else:
                    nc.vector.tensor_tensor(
                        tmp[:], xtap(t), wt[:], mybir.AluOpType.mult
                    )
                    nc.vector.tensor_tensor(
                        acc[:], acc[:], tmp[:], mybir.AluOpType.add
                    )
        nc.scalar.dma_start(out=ov[:, r0 : r0 + rt, :], in_=acc[:])


def build_program(rep=1, na=None, mode="g32", rt=None, split=False):
    nc = bacc.Bacc(
        "TRN2",
        target_bir_lowering=False,
        debug=False,
        enable_asserts=False,
        num_devices=8,
    )
    xs = nc.dram_tensor("xs", [128, SLAB_R * SLAB_C], _F32, kind="ExternalInput").ap()
    w = nc.dram_tensor("w", [C, KH, KW, H, W], _F32, kind="ExternalInput").ap()
    o = nc.dram_tensor("o", [C, H, W], _F32, kind="ExternalOutput").ap()
    with TileContext(nc) as tc:
        _emit(nc, tc, xs, w, o, rep=rep, na=na, mode=mode, rt=rt, split=split)
    nc.compile()
    return nc


def make_slab(x_one):
    """Host-side zero-padded slab for one sample: [64,128,128] -> [128, 66*130].

    Partition p = c*2 + hf holds rows hf*64-1 .. hf*64+64 of channel c
    (zero-padded at the image border) in a 66x130 col-padded layout.
    """
    slab = np.zeros((C, 2, SLAB_R, SLAB_C), dtype=np.float32)
    # half 0: slab rows 1..65 <- x rows 0..64 (row 0 stays zero: top pad)
    slab[:, 0, 1 : HALF + 2, 1 : W + 1] = x_one[:, 0 : HALF + 1, :]
    # half 1: slab rows 0..64 <- x rows 63..127 (row 65 stays zero: bottom pad)
    slab[:, 1, 0 : HALF + 1, 1 : W + 1] = x_one[:, HALF - 1 : H, :]
    return slab.reshape(128, SLAB_R * SLAB_C)


_CACHE = {}


def kernel(input, weight, _trace=False):
    input = np.asarray(input, dtype=np.float32)
    weight = np.asarray(weight, dtype=np.float32)
    assert input.shape == (B, C, H, W), input.shape
    assert weight.shape == (B, C, KH, KW, H, W), weight.shape

    if "nc" not in _CACHE:
        _CACHE["nc"] = build_program()
    nc = _CACHE["nc"]

    in_maps = [
        {"xs": make_slab(input[b]), "w": np.ascontiguousarray(weight[b])}
        for b in range(B)
    ]
    res = run_bass_kernel_spmd(nc, in_maps, core_ids=list(range(B)), trace=_trace)
    _CACHE["last_result"] = res
    out = np.stack([res.results[b]["o"] for b in range(B)], axis=0)
    return out.astype(np.float32, copy=False)

